# revision 1
# baseline (speedup 1.0000x reference)
"""Trainium2 Bass kernel for nn_BRNNIntegrateOnehot.

Reference computation (per batch b):
    h = one_hot(0, S)
    for t in 0..L-1:
        h = clip(h @ fsa[input[b, t]], -10, 10)
        out[b, t, :] = h

Data-parallel over batch B across 8 cores (8 rows each), fsa replicated.
Per (b, t) the 64KB matrix fsa[tok] is gathered on-device with one
indirect DMA (per-partition offsets tok*128+p pull matrix row p onto
partition p -> lhsT layout). The mat-vec is one f32 PE matmul
(lhsT=T, rhs=h column), clip is a fused max/min tensor_scalar on DVE, and
the h history is transposed at the end with DVE 32x32 block transposes for
contiguous output stores.

Hardware notes (measured): walrus's dynamic DMA consumes exactly one
offset per destination partition with a contiguous run each, so one
indirect DMA moves one matrix; SWDGE per-op overhead makes the gather
stream run at ~46 GB/s/core. Faster paths (multi-index dma_gather,
transpose-gather of bf16 planes at 342 GB/s) were prototyped but the PE
LDWEIGHTS path requires contiguous weight columns, which those layouts
cannot provide without a repack that costs more than it saves; correctness
won over peak bandwidth for this submission.

Raw bass (explicit engine programs + semaphores). Self-contained.
"""

import numpy as np

V, S = 10000, 128
B, L = 64, 512
N_CORES = 8
B_LOC = B // N_CORES  # 8


def build_kernel(l=L, b_loc=B_LOC, v=V, g_slots=64, instrument=False,
                 tick_cyc=12000, maxtick=1024):
    import concourse.bass as bass
    from concourse import mybir
    from contextlib import ExitStack

    f32 = mybir.dt.float32
    t_blk = 8
    assert l % t_blk == 0
    n_blk = l // t_blk
    tsz = min(l, 128)
    assert l % tsz == 0 and tsz % 32 == 0
    n_band = l // tsz
    n_psum = 4
    n_mat = l * b_loc

    NQ = 4
    nc = bass.Bass("TRN2", num_swdge_queues=NQ)
    fsa = nc.dram_tensor("fsa", [v * S, S], f32, kind="ExternalInput")
    offs = nc.dram_tensor("offs", [128, n_mat], mybir.dt.int32, kind="ExternalInput")
    out = nc.dram_tensor("out", [b_loc, l, S], f32, kind="ExternalOutput")
    if instrument:
        mark_d = nc.dram_tensor("marker_out", [1, maxtick], f32, kind="ExternalOutput")

    with ExitStack() as stack:
        offs_sb = stack.enter_context(
            nc.sbuf_tensor("offs_sb", [128, n_mat], mybir.dt.int32))
        h_hist = stack.enter_context(nc.sbuf_tensor("h_hist", [128, l, b_loc], f32))
        h0 = stack.enter_context(nc.sbuf_tensor("h0", [128, 1], f32))
        gbuf = stack.enter_context(nc.sbuf_tensor("gbuf", [128, g_slots, S], f32))
        stbuf = stack.enter_context(nc.sbuf_tensor("stbuf", [128, 4, 128], f32))
        ph = stack.enter_context(nc.psum_tensor("ph", [128, n_psum, 512], f32))
        offs_sem = stack.enter_context(nc.semaphore("offs_sem"))
        dsems = [stack.enter_context(nc.semaphore(f"d{i}")) for i in range(NQ)]
        dve_sem = stack.enter_context(nc.semaphore("dve_sem"))
        pe_h_sem = stack.enter_context(nc.semaphore("pe_h_sem"))
        tr_sem = stack.enter_context(nc.semaphore("tr_sem"))
        so_sem = stack.enter_context(nc.semaphore("so_sem"))
        if instrument:
            marker = stack.enter_context(nc.sbuf_tensor("marker", [1, maxtick], f32))
            mk_sem = stack.enter_context(nc.semaphore("mk_sem"))
            ms_sem = stack.enter_context(nc.semaphore("ms_sem"))
        block = stack.enter_context(nc.Block())

        n_out_dma = b_loc * n_band

        @block.sync
        def _(sync):
            sync.dma_start(out=offs_sb[:, :], in_=offs[:, :]).then_inc(offs_sem, 16)
            i = 0
            for b in range(b_loc):
                for tb in range(n_band):
                    sync.wait_ge(tr_sem, i + 1)
                    sync.dma_start(
                        out=out[b, tb * tsz : (tb + 1) * tsz, :],
                        in_=stbuf[:tsz, i % 4, :],
                    ).then_inc(so_sem, 16)
                    i += 1
            if instrument:
                sync.wait_ge(so_sem, 16 * n_out_dma)
                sync.wait_ge(mk_sem, 1)
                sync.dma_start(out=mark_d[:, :], in_=marker[:, :]).then_inc(ms_sem, 16)

        @block.gpsimd
        def _(gpsimd):
            gpsimd.wait_ge(offs_sem, 16)
            for n in range(n_mat):
                if n >= g_slots:
                    # slot reuse: consumed when its step finished
                    gpsimd.wait_ge(pe_h_sem, (n - g_slots) // b_loc + 1)
                qi = n % NQ
                d = gpsimd.indirect_dma_start(
                    out=gbuf[:, n % g_slots, :],
                    out_offset=None,
                    in_=fsa[:],
                    in_offset=bass.IndirectOffsetOnAxis(
                        ap=offs_sb[:, n : n + 1], axis=0
                    ),
                )
                # round-robin the gather stream over the 4 SWDGE queues;
                # per-queue FIFO keeps each dsems[qi] ordering sound
                # (b_loc % NQ == 0 -> exactly b_loc/NQ ops per queue per step).
                d.ins.queue = f"qPoolDynamic{qi or ''}"
                d.then_inc(dsems[qi], 16)

        @block.tensor
        def _(tensor):
            per_q = b_loc // NQ
            for t in range(l):
                for qi in range(NQ):
                    tensor.wait_ge(dsems[qi], 16 * per_q * (t + 1))
                tensor.wait_ge(dve_sem, t + 1)
                mm = None
                for b in range(b_loc):
                    n = t * b_loc + b
                    rhs = h0[:, 0:1] if t == 0 else h_hist[:, t - 1, b : b + 1]
                    mm = tensor.matmul(
                        out=ph[:, t % n_psum, b : b + 1],
                        lhsT=gbuf[:, n % g_slots, :],
                        rhs=rhs,
                        start=True,
                        stop=True,
                    )
                mm.then_inc(pe_h_sem, 1)

        @block.vector
        def _(vector):
            vector.memset(h0[:, :], 0.0)
            vector.memset(h0[:1, :], 1.0).then_inc(dve_sem, 1)
            if instrument:
                vector.memset(marker[:, :], 0.0).then_inc(mk_sem, 1)
            for t in range(l):
                vector.wait_ge(pe_h_sem, t + 1)
                vector.tensor_scalar(
                    h_hist[:, t, :],
                    ph[:, t % n_psum, 0:b_loc],
                    -10.0,
                    10.0,
                    mybir.AluOpType.max,
                    mybir.AluOpType.min,
                ).then_inc(dve_sem, 1)
            i = 0
            for b in range(b_loc):
                for tb in range(n_band):
                    if i >= 4:
                        vector.wait_ge(so_sem, 16 * (i - 3))
                    tr = None
                    for jb in range(tsz // 32):
                        for ib in range(4):
                            tr = vector.transpose(
                                out=stbuf[
                                    32 * jb : 32 * (jb + 1),
                                    i % 4,
                                    32 * ib : 32 * (ib + 1),
                                ],
                                in_=h_hist[
                                    32 * ib : 32 * (ib + 1),
                                    tb * tsz + 32 * jb : tb * tsz + 32 * (jb + 1),
                                    b,
                                ],
                            )
                    tr.then_inc(tr_sem, 1)
                    i += 1

        if instrument:

            @block.scalar
            def _(scalar):
                scalar.wait_ge(offs_sem, 16)
                for i in range(maxtick):
                    scalar.nop(cycle_cnt=tick_cyc, nofuse=True)
                    scalar.add(marker[:1, i : i + 1], marker[:1, i : i + 1], 1.0)

    return nc


def make_offs(tok_c, s=S):
    """tok_c: [b_loc, l] ints -> offs [128, l*b_loc] int32, col = t*b_loc + b;
    offs[p, c] = tok*128 + p (per-partition row index into fsa [V*S, S])."""
    base = (tok_c.T.astype(np.int64) * s).reshape(1, -1)  # t-major, b-minor
    return (base + np.arange(s, dtype=np.int64).reshape(s, 1)).astype(np.int32)


def kernel(input, lengths, fsa_tensor):
    from concourse.bass_utils import run_bass_kernel_spmd

    tok = np.asarray(input)
    fsa = np.ascontiguousarray(
        np.asarray(fsa_tensor, dtype=np.float32).reshape(V * S, S)
    )
    nc = build_kernel()
    in_maps = []
    for c in range(N_CORES):
        tok_c = tok[c * B_LOC : (c + 1) * B_LOC]
        in_maps.append({"fsa": fsa, "offs": make_offs(tok_c)})
    res = run_bass_kernel_spmd(nc, in_maps, core_ids=list(range(N_CORES)))
    return np.concatenate([r["out"] for r in res.results], axis=0)



# revision 20
# speedup vs baseline: 2.2500x; 2.2500x over previous
"""Trainium2 Bass kernel for nn_BRNNIntegrateOnehot.

Reference computation (per batch b):
    h = one_hot(0, S)
    for t in 0..L-1:
        h = clip(h @ fsa[input[b, t]], -10.0, 10.0)
        out[b, t, :] = h

Data-parallel over batch B across 8 cores (8 sequences each), fsa
replicated per core in HBM as a bf16 table.  The per-(b, t) 32KB matrix
gather runs as a *register-addressed HWDGE* DMA: the host precomputes the
byte offset tok*S*S*2 for every stream position, the SP engine bulk-loads
8 offsets per TensorLoad into registers and issues one hardware-DGE
dma_start per matrix with the register as the DRAM base (the SWDGE ~2us
per-op fixed cost of indirect DMA is what limited the previous version to
~46 GB/s/core).  The mat-vec is one bf16 PE matmul per lane (lhsT = T so
h stays a [128,1] column; FWL halves the weight-load time vs f32), clip
is a fused max/min tensor_scalar on DVE writing bf16, and the h history
is transposed band-by-band with DVE 32x32 block transposes trickled one
per step so output stores (on ACT) overlap the scan.

Output returns as bf16 and is upcast on the host; the |err|/max|expected|
metric is ~1e-3 dominated by bf16 rounding of the t=0 rows.

Raw bass (explicit engine programs + semaphores).  Self-contained.
"""

import numpy as np

V, S = 10000, 128
B, L = 64, 512
N_CORES = 8
B_LOC = B // N_CORES  # 8

TBYTES = 2             # fsa table element size: 2 = bf16, 1 = fp8 e4m3
MAT_BYTES = S * S * TBYTES
ROW_BYTES = S * TBYTES


def build_kernel(l=L, b_loc=B_LOC, v=V, g_slots=512, tsz=64, instrument=False,
                 tick_cyc=12000, maxtick=256):
    import concourse.bass as bass
    from concourse import mybir
    from concourse.bass_types import AP
    from contextlib import ExitStack

    f32 = mybir.dt.float32
    bf16 = mybir.dt.bfloat16
    i8 = mybir.dt.int8
    i32 = mybir.dt.int32

    n_mat = l * b_loc
    assert l % tsz == 0 and tsz % 32 == 0
    n_band = l // tsz
    n_grp = n_band * b_loc          # output DMA groups (band-major)
    n_psum = 4
    LOADW = 32                      # offsets per TensorLoad

    nc = bass.Bass("TRN2")
    fsa8 = nc.dram_tensor("fsa8", [v * S, ROW_BYTES], i8, kind="ExternalInput")
    offs = nc.dram_tensor("offs", [1, n_mat], i32, kind="ExternalInput")
    out = nc.dram_tensor("out", [b_loc, l, S], bf16, kind="ExternalOutput")
    if instrument:
        mark_d = nc.dram_tensor("marker_out", [1, maxtick], f32, kind="ExternalOutput")

    tab_dt = bf16 if TBYTES == 2 else mybir.dt.float8e4

    with ExitStack() as stack:
        offs_sb = stack.enter_context(nc.sbuf_tensor("offs_sb", [1, n_mat], i32))
        gbuf = stack.enter_context(nc.sbuf_tensor("gbuf", [128, g_slots, S], tab_dt))
        gbuf8 = gbuf.bitcast(i8)    # [128, g_slots, ROW_BYTES]
        h_hist = stack.enter_context(nc.sbuf_tensor("h_hist", [128, l, b_loc], bf16))
        h0 = stack.enter_context(nc.sbuf_tensor("h0", [128, 1], bf16))
        # one stbuf slot per output group: DVE never waits on output DMAs
        # (waiting would deadlock — sync only drains outputs after the
        # gather loop, which is gated on PE progress, which needs DVE)
        stbuf = stack.enter_context(nc.sbuf_tensor("stbuf", [tsz, n_grp, S], bf16))
        ph = stack.enter_context(nc.psum_tensor("ph", [128, n_psum, 512], f32))
        offs_sem = stack.enter_context(nc.semaphore("offs_sem"))
        gsems = [stack.enter_context(nc.semaphore(f"gsem{r}")) for r in range(4)]
        pe_h_sem = stack.enter_context(nc.semaphore("pe_h_sem"))
        dve_sem = stack.enter_context(nc.semaphore("dve_sem"))
        tr_sem = stack.enter_context(nc.semaphore("tr_sem"))
        so_sem = stack.enter_context(nc.semaphore("so_sem"))
        if instrument:
            marker = stack.enter_context(nc.sbuf_tensor("marker", [1, maxtick], f32))
            mk_sem = stack.enter_context(nc.semaphore("mk_sem"))
            ms_sem = stack.enter_context(nc.semaphore("ms_sem"))
        block = stack.enter_context(nc.Block())

        row_ap = fsa8[0:S, :].ap    # [[ROW_BYTES, 128], [1, ROW_BYTES]]

        @block.sync
        def _(sync):
            from concourse.expressions import RuntimeValue

            sync.dma_start(out=offs_sb[:, :], in_=offs[:, :]).then_inc(offs_sem, 16)
            sync.wait_ge(offs_sem, 16)
            regs = [sync.alloc_register(f"g{i}") for i in range(LOADW)]
            for n in range(n_mat):
                if n % LOADW == 0:
                    sync.load(regs, offs_sb[0:1, n : n + LOADW])
                if n >= g_slots and (n - g_slots) % b_loc == 0:
                    # slot reuse: wait until the step that consumed this
                    # slot's previous occupant has finished on PE
                    sync.wait_ge(pe_h_sem, (n - g_slots) // b_loc + 1)
                vb = RuntimeValue(
                    regs[n % LOADW], min_val=0, max_val=(v - 1) * MAT_BYTES
                )
                sap = AP(tensor=fsa8, offset=vb, ap=row_ap, dep_tracking_offset=0)
                # dynamic-DMA ops must each carry sync info; rotate over 4
                # sems so no counter exceeds 16*4096/4 (16-bit safe)
                sync.dma_start(out=gbuf8[:, n % g_slots, :], in_=sap).then_inc(
                    gsems[(n // b_loc) % 4], 16
                )
            for g in range(n_grp):
                b = g % b_loc
                tb = g // b_loc
                sync.wait_ge(tr_sem, g + 1)
                sync.dma_start(
                    out=out[b, tb * tsz : (tb + 1) * tsz, :],
                    in_=stbuf[:, g, :],
                ).then_inc(so_sem, 16)
            if instrument:
                sync.wait_ge(so_sem, 16 * n_grp)
                sync.wait_ge(mk_sem, 1)
                sync.dma_start(out=mark_d[:, :], in_=marker[:, :]).then_inc(ms_sem, 16)

        if instrument:

            @block.scalar
            def _(scalar):
                scalar.wait_ge(offs_sem, 16)
                for i in range(maxtick):
                    scalar.nop(cycle_cnt=tick_cyc, nofuse=True)
                    scalar.add(marker[:1, i : i + 1], marker[:1, i : i + 1], 1.0)

        @block.tensor
        def _(tensor):
            for t in range(l):
                tensor.wait_ge(gsems[t % 4], 16 * b_loc * (t // 4 + 1))
                tensor.wait_ge(dve_sem, t + 1)
                mm = None
                for b in range(b_loc):
                    n = t * b_loc + b
                    rhs = h0[:, 0:1] if t == 0 else h_hist[:, t - 1, b : b + 1]
                    mm = tensor.matmul(
                        out=ph[:, t % n_psum, b : b + 1],
                        lhsT=gbuf[:, n % g_slots, :],
                        rhs=rhs,
                        start=True,
                        stop=True,
                    )
                mm.then_inc(pe_h_sem, 1)

        @block.vector
        def _(vector):
            vector.memset(h0[:, :], 0.0)
            vector.memset(h0[:1, :], 1.0).then_inc(dve_sem, 1)
            if instrument:
                vector.memset(marker[:, :], 0.0).then_inc(mk_sem, 1)

            n_jb = tsz // 32        # 32-blocks along t within a band
            n_ib = S // 32          # 32-blocks along state dim
            per_band = b_loc * n_jb * n_ib  # == tsz when tsz==64,b_loc==8? no:
            # b_loc * n_jb * n_ib = 8*2*4 = 64 == tsz steps per band: 1/step

            def do_transpose(tb, k):
                # k in [0, per_band): block index within band tb
                b = k // (n_jb * n_ib)
                jb = (k % (n_jb * n_ib)) // n_ib
                ib = k % n_ib
                g = tb * b_loc + b
                tr = vector.transpose(
                    out=stbuf[32 * jb : 32 * (jb + 1), g, 32 * ib : 32 * (ib + 1)],
                    in_=h_hist[
                        32 * ib : 32 * (ib + 1),
                        tb * tsz + 32 * jb : tb * tsz + 32 * (jb + 1),
                        b,
                    ],
                )
                if k % (n_jb * n_ib) == n_jb * n_ib - 1:
                    tr.then_inc(tr_sem, 1)

            for t in range(l):
                vector.wait_ge(pe_h_sem, t + 1)
                vector.tensor_scalar(
                    h_hist[:, t, :],
                    ph[:, t % n_psum, 0:b_loc],
                    -10.0,
                    10.0,
                    mybir.AluOpType.max,
                    mybir.AluOpType.min,
                ).then_inc(dve_sem, 1)
                # trickle previous band's transposes, one per step
                tb = t // tsz - 1
                if tb >= 0:
                    do_transpose(tb, t % tsz)
            for k in range(per_band):
                do_transpose(n_band - 1, k)

    return nc


def make_offs(tok_c):
    """tok_c: [b_loc, l] ints -> [1, l*b_loc] int32 byte offsets into the
    bf16 fsa table, stream order n = t*b_loc + b."""
    return (tok_c.T.astype(np.int64) * MAT_BYTES).reshape(1, -1).astype(np.int32)


def _prep_fsa(fsa_tensor):
    import ml_dtypes

    np_dt = ml_dtypes.bfloat16 if TBYTES == 2 else ml_dtypes.float8_e4m3
    fsa_t = np.asarray(fsa_tensor, dtype=np.float32).astype(np_dt)
    return np.ascontiguousarray(fsa_t).view(np.int8).reshape(V * S, ROW_BYTES)


def run(input, lengths, fsa_tensor):
    from concourse.bass_utils import run_bass_kernel_spmd

    tok = np.asarray(input)
    fsa8 = _prep_fsa(fsa_tensor)
    nc = build_kernel()
    in_maps = []
    for c in range(N_CORES):
        tok_c = tok[c * B_LOC : (c + 1) * B_LOC]
        in_maps.append({"fsa8": fsa8, "offs": make_offs(tok_c)})
    res = run_bass_kernel_spmd(nc, in_maps, core_ids=list(range(N_CORES)))
    out = np.concatenate(
        [r["out"].astype(np.float32) for r in res.results], axis=0
    )
    return out, res


def kernel(input, lengths, fsa_tensor):
    out, _ = run(input, lengths, fsa_tensor)
    return out


# revision 26
# speedup vs baseline: 6.6661x; 2.9627x over previous
"""Trainium2 Bass kernel for nn_BRNNIntegrateOnehot.

Reference computation (per batch b):
    h = one_hot(0, S)
    for t in 0..L-1:
        h = clip(h @ fsa[input[b, t]], -10.0, 10.0)
        out[b, t, :] = h

Data-parallel over batch B across 8 cores (8 sequences each), fsa
replicated per core in HBM as a bf16 table.  The dominant cost is the
per-(b, t) 32KB matrix gather (2.1GB of table traffic total).  Gathers
run as *static pair ops*: token ids are known on the host when kernel()
is called, so the program bakes the byte offsets in, coalescing two
stream-adjacent matrices into one 3-dim-AP HWDGE dma_start (any two
addresses form a 2-element stride); this amortizes the ~0.6us per-op
fixed cost of dynamic-HWDGE descriptor generation that limits
one-matrix-per-op designs.  Ops are split across the SP and ACT engines
(two parallel HW-DGE rings, ~1.85x).  Since each core's token stream
differs, the single SPMD program carries one gather stream per core,
branched on partition_id() (all branches have identical semaphore/slot
schedules, so the shared PE/DVE programs work for every core).

The mat-vec is one bf16 PE matmul per lane (lhsT = T so h stays a
[128,1] column; FWL halves weight-load time vs f32), clip is a fused
max/min tensor_scalar on DVE writing bf16, and the h history is
transposed band-by-band with DVE 32x32 block transposes trickled one per
step.  Output stores drain on SP after the gather stream ends (stbuf has
one slot per output group, so DVE never blocks on them).

Output returns as bf16 and is upcast on the host; the
max|err|/max|expected| metric is ~2e-4, dominated by bf16 rounding.

Raw bass (explicit engine programs + semaphores).  Self-contained.
"""

import numpy as np

V, S = 10000, 128
B, L = 64, 512
N_CORES = 8
B_LOC = B // N_CORES  # 8

TBYTES = 2             # fsa table element size: 2 = bf16, 1 = fp8 e4m3
MAT_BYTES = S * S * TBYTES
ROW_BYTES = S * TBYTES


def build_kernel(offs_cores, l=L, b_loc=B_LOC, v=V, g_slots=512, tsz=64,
                 instrument=False, tick_cyc=12000, maxtick=256):
    """offs_cores: [n_cores][l*b_loc] int byte offsets into the table,
    stream order n = t*b_loc + b."""
    import concourse.bass as bass
    from concourse import mybir
    from concourse.bass_types import AP
    from contextlib import ExitStack

    f32 = mybir.dt.float32
    bf16 = mybir.dt.bfloat16
    i8 = mybir.dt.int8

    n_cores = len(offs_cores)
    n_mat = l * b_loc
    n_pair = n_mat // 2
    assert l % tsz == 0 and tsz % 32 == 0
    assert g_slots % 4 == 0
    n_band = l // tsz
    n_grp = n_band * b_loc          # output DMA groups (band-major)
    n_psum = 4

    nc = bass.Bass("TRN2")
    fsa8 = nc.dram_tensor("fsa8", [v * S, ROW_BYTES], i8, kind="ExternalInput")
    out = nc.dram_tensor("out", [b_loc, l, S], bf16, kind="ExternalOutput")
    if instrument:
        mark_d = nc.dram_tensor("marker_out", [1, maxtick], f32, kind="ExternalOutput")

    tab_dt = bf16 if TBYTES == 2 else mybir.dt.float8e4

    with ExitStack() as stack:
        gbuf = stack.enter_context(nc.sbuf_tensor("gbuf", [128, g_slots, S], tab_dt))
        gbuf8 = gbuf.bitcast(i8)    # [128, g_slots, ROW_BYTES]
        h_hist = stack.enter_context(nc.sbuf_tensor("h_hist", [128, l, b_loc], bf16))
        h0 = stack.enter_context(nc.sbuf_tensor("h0", [128, 1], bf16))
        # one stbuf slot per output group: DVE never waits on output DMAs
        stbuf = stack.enter_context(nc.sbuf_tensor("stbuf", [tsz, n_grp, S], bf16))
        ph = stack.enter_context(nc.psum_tensor("ph", [128, n_psum, 512], f32))
        # 8 rotating completion sems per gather engine: a wait on class k%8
        # at count k//8+1 plus per-SDMA-engine ring-FIFO order proves every
        # op <= k fully landed (a plain summed sem can transiently reach
        # 16*K with op K still in flight when engines skew by one op)
        NGS = 8
        sp_gsems = [
            stack.enter_context(nc.semaphore(f"sp_gsem{r}")) for r in range(NGS)
        ]
        act_gsems = [
            stack.enter_context(nc.semaphore(f"act_gsem{r}")) for r in range(NGS)
        ]
        pe_h_sem = stack.enter_context(nc.semaphore("pe_h_sem"))
        dve_sem = stack.enter_context(nc.semaphore("dve_sem"))
        tr_sem = stack.enter_context(nc.semaphore("tr_sem"))
        so_sem = stack.enter_context(nc.semaphore("so_sem"))
        if instrument:
            marker = stack.enter_context(nc.sbuf_tensor("marker", [1, maxtick], f32))
            ms_sem = stack.enter_context(nc.semaphore("ms_sem"))
        block = stack.enter_context(nc.Block())

        def pair_in_ap(o1, o2):
            """3-dim AP covering matrices at byte offsets o1, o2 (in stream
            order); returns (ap, reversed)."""
            lo, hi, rev = (o1, o2, False) if o2 >= o1 else (o2, o1, True)
            return AP(
                tensor=fsa8,
                offset=int(lo),
                ap=[[ROW_BYTES, S], [int(hi - lo), 2], [1, ROW_BYTES]],
                dep_tracking_offset=0,
            ), rev

        def pair_out_ap(s, rev):
            if not rev:
                return gbuf8[:, s : s + 2, :]
            return AP(
                tensor=gbuf8,
                offset=(s + 1) * ROW_BYTES,
                ap=[[g_slots * ROW_BYTES, 128], [-ROW_BYTES, 2], [1, ROW_BYTES]],
                dep_tracking_offset=0,
            )

        def gather_stream(eng, sems, parity):
            """Pair ops j = parity, parity+2, ... on engine eng."""
            pid = eng.partition_id()
            for c in range(n_cores):
                offs = offs_cores[c]
                with eng.If(pid == c):
                    for k in range(n_pair // 2):
                        j = 2 * k + parity
                        p0 = 2 * j          # first stream position
                        if p0 + 1 >= g_slots:
                            eng.wait_ge(pe_h_sem, (p0 + 1 - g_slots) // 8 + 1)
                        sap, rev = pair_in_ap(offs[p0], offs[p0 + 1])
                        eng.dma_start(
                            out=pair_out_ap(p0 % g_slots, rev), in_=sap
                        ).then_inc(sems[k % NGS], 16)

        @block.sync
        def _(sync):
            gather_stream(sync, sp_gsems, 0)
            for g in range(n_grp):
                b = g % b_loc
                tb = g // b_loc
                sync.wait_ge(tr_sem, g + 1)
                sync.dma_start(
                    out=out[b, tb * tsz : (tb + 1) * tsz, :],
                    in_=stbuf[:, g, :],
                ).then_inc(so_sem, 16)
            if instrument:
                sync.wait_ge(so_sem, 16 * n_grp)
                sync.dma_start(out=mark_d[:, :], in_=marker[:, :]).then_inc(
                    ms_sem, 16
                )

        @block.scalar
        def _(scalar):
            gather_stream(scalar, act_gsems, 1)

        @block.tensor
        def _(tensor):
            for t in range(l):
                # last gather op needed for step t is k = 2t+1 on each engine
                k = 2 * t + 1
                tensor.wait_ge(sp_gsems[k % NGS], 16 * (k // NGS + 1))
                tensor.wait_ge(act_gsems[k % NGS], 16 * (k // NGS + 1))
                tensor.wait_ge(dve_sem, t + 1)
                mm = None
                for b in range(b_loc):
                    n = t * b_loc + b
                    rhs = h0[:, 0:1] if t == 0 else h_hist[:, t - 1, b : b + 1]
                    mm = tensor.matmul(
                        out=ph[:, t % n_psum, b : b + 1],
                        lhsT=gbuf[:, n % g_slots, :],
                        rhs=rhs,
                        start=True,
                        stop=True,
                    )
                mm.then_inc(pe_h_sem, 1)

        @block.vector
        def _(vector):
            vector.memset(h0[:, :], 0.0)
            vector.memset(h0[:1, :], 1.0).then_inc(dve_sem, 1)

            n_jb = tsz // 32        # 32-blocks along t within a band
            n_ib = S // 32          # 32-blocks along state dim
            per_band = b_loc * n_jb * n_ib  # == tsz steps per band: 1/step

            def do_transpose(tb, k):
                b = k // (n_jb * n_ib)
                jb = (k % (n_jb * n_ib)) // n_ib
                ib = k % n_ib
                g = tb * b_loc + b
                tr = vector.transpose(
                    out=stbuf[32 * jb : 32 * (jb + 1), g, 32 * ib : 32 * (ib + 1)],
                    in_=h_hist[
                        32 * ib : 32 * (ib + 1),
                        tb * tsz + 32 * jb : tb * tsz + 32 * (jb + 1),
                        b,
                    ],
                )
                if k % (n_jb * n_ib) == n_jb * n_ib - 1:
                    tr.then_inc(tr_sem, 1)

            for t in range(l):
                vector.wait_ge(pe_h_sem, t + 1)
                vector.tensor_scalar(
                    h_hist[:, t, :],
                    ph[:, t % n_psum, 0:b_loc],
                    -10.0,
                    10.0,
                    mybir.AluOpType.max,
                    mybir.AluOpType.min,
                ).then_inc(dve_sem, 1)
                tb = t // tsz - 1
                if tb >= 0:
                    do_transpose(tb, t % tsz)
            for k in range(per_band):
                do_transpose(n_band - 1, k)

        if instrument:

            @block.gpsimd
            def _(gpsimd):
                gpsimd.memset(marker[:, :], 0.0)
                for i in range(maxtick):
                    gpsimd.nop(cycle_cnt=tick_cyc, nofuse=True)
                    gpsimd.memset(marker[:1, i : i + 1], 1.0)

    return nc


def make_offs(tok_c):
    """tok_c: [b_loc, l] ints -> flat [l*b_loc] int64 byte offsets, stream
    order n = t*b_loc + b."""
    return (tok_c.T.astype(np.int64) * MAT_BYTES).reshape(-1)


def _prep_fsa(fsa_tensor):
    import ml_dtypes

    np_dt = ml_dtypes.bfloat16 if TBYTES == 2 else ml_dtypes.float8_e4m3
    fsa_t = np.asarray(fsa_tensor, dtype=np.float32).astype(np_dt)
    return np.ascontiguousarray(fsa_t).view(np.int8).reshape(V * S, ROW_BYTES)


def run(input, lengths, fsa_tensor):
    from concourse.bass_utils import run_bass_kernel_spmd

    tok = np.asarray(input)
    fsa8 = _prep_fsa(fsa_tensor)
    offs_cores = [
        make_offs(tok[c * B_LOC : (c + 1) * B_LOC]) for c in range(N_CORES)
    ]
    nc = build_kernel(offs_cores)
    in_maps = [{"fsa8": fsa8} for _ in range(N_CORES)]
    res = run_bass_kernel_spmd(nc, in_maps, core_ids=list(range(N_CORES)))
    out = np.concatenate(
        [r["out"].astype(np.float32) for r in res.results], axis=0
    )
    return out, res


def kernel(input, lengths, fsa_tensor):
    out, _ = run(input, lengths, fsa_tensor)
    return out
